# revision 18
# baseline (speedup 1.0000x reference)
"""CapsuleLayer dynamic-routing kernel for 8 Trainium2 NeuronCores, v3.

Math (reference):
    u_hat[b,n,j,d] = sum_i W[n,j,d,i] * x[b,j,i]
    b = 0; for r in 0..2:
        c = softmax_n(b); s[b,n,d] = sum_j c*u_hat; v = squash_d(s)
        if r < 2: b += sum_d v*u_hat
    return v  [B, N, D]

Design: u_hat is NEVER materialized (its per-j matmul only uses K=16 of
the 128 PE rows). All big PE work runs at K=128:

  r0:   s0[b,(n,d)] = (1/N) sum_{(j,i)} x[b,(j,i)] W[(j,i),(n,d)]
        -> dense matmuls, stationary x K-tiles [(8j,16i),b].
  r>=1: dlog[b,n,j] = sum_i A[b,n,j,i] x[b,j,i],
        A[b,n,j,i] = sum_d v[b,n,d] W[n,j,d,i] computed by PE with a
        block-diagonal stationary blockdiag(v_n^T for 4 n) [128,128]
        and moving W_A[(q,d),(j,i)] -> out [(q,b),(j,i)] at K=128.
        The i-reduction is a DVE mult + add-tree (L1 on GpSimd).
  r>=1: s via y = c (x) x folded: stationary W_S[(j),(c8:q,d)] chunks,
        moving y^T[(j),(q,b)] computed directly in j-partition layout
        from a PE-transposed c. Diagonal 4x4 blocks of the [128,128]
        outputs are the s partials, accumulated in PSUM over (i, jt).
        PSUM accumulators are memset-initialized and all matmuls use
        start=False (start=True resets the whole PSUM bank, which
        clobbers co-located accumulator regions).

Pipelining: softmax over n only needs all capsules for a FIXED j, so
the iteration is split into j-halves (jt): while the s-matmuls of
jt=0 run on the PE, the A-phase of jt=1 runs concurrently (its DVE /
GpSimd reduction work hides under the PE stream). A tiny warmup
AllReduce absorbs the collective's ring-setup cost during r0's weight
streaming, and big weight DMAs are spread over several engine queues.

Sharding: J (2048) split 8 ways -> Jc=256/core; s AllReduced in f32.
"""

import functools
import numpy as np

B, J, I = 32, 2048, 16
N, D = 64, 32
NCORES = 8
JC = J // NCORES          # 256 j per core
ND = N * D                # 2048
NCH = 16                  # n-blocks of 4 (ch); n = 4*ch + q
ROUTINGS = 3
EPS = 1e-7


@functools.lru_cache(maxsize=2)
def _build(dbg=False):
    import concourse.bass as bass
    import concourse.mybir as mybir
    import concourse.bacc as bacc
    import concourse.tile as tile

    f32 = mybir.dt.float32
    bf16 = mybir.dt.bfloat16
    MUL = mybir.AluOpType.mult
    ADD = mybir.AluOpType.add
    AX = mybir.AxisListType.X
    AF = mybir.ActivationFunctionType

    nc = bacc.Bacc("TRN2", target_bir_lowering=False, debug=False,
                   num_devices=NCORES)

    # --- DRAM inputs (per-core shard layouts, prepared host-side) ---
    # r0: wm[cch, g, (8jr,16i), (4tt,512)]  (t = 4g+tt)
    wm_d = nc.dram_tensor("wm", [4, 8, 128, 2048], bf16, kind="ExternalInput")
    xm_d = nc.dram_tensor("xm", [128, 32, B], bf16, kind="ExternalInput")
    wa_d = nc.dram_tensor("wa", [NCH, 128, JC * I], bf16, kind="ExternalInput")
    xa_d = nc.dram_tensor("xa", [128, JC * I], bf16, kind="ExternalInput")
    # s-phase: ws[chB, jt, ig, (j), (i2,c8,q,d)]  (i = 2*ig+i2)
    ws_d = nc.dram_tensor("ws", [2, 2, 8, 128, 2048], bf16,
                          kind="ExternalInput")
    # xq[(j), jt, i, (q,b)] : x replicated over q
    xq_d = nc.dram_tensor("xq", [128, 2, I, 128], bf16, kind="ExternalInput")
    id_d = nc.dram_tensor("idm", [128, 128], bf16, kind="ExternalInput")
    idf_d = nc.dram_tensor("idf", [128, 128], f32, kind="ExternalInput")
    sel_d = nc.dram_tensor("sel", [128, B], bf16, kind="ExternalInput")
    rep_d = nc.dram_tensor("rep", [B, 128], bf16, kind="ExternalInput")
    ond_d = nc.dram_tensor("ond", [128, 4], f32, kind="ExternalInput")
    rp4_d = nc.dram_tensor("rp4", [4, 128], f32, kind="ExternalInput")
    v_d = nc.dram_tensor("v", [B, ND], f32, kind="ExternalOutput")
    if dbg:
        dbg_s0 = nc.dram_tensor("dbg_s0", [128, NCH * B], f32,
                                kind="ExternalOutput")
        dbg_lg = nc.dram_tensor("dbg_lg", [128, NCH * 256], bf16,
                                kind="ExternalOutput")
        dbg_sp = nc.dram_tensor("dbg_sp", [128, NCH * B], f32,
                                kind="ExternalOutput")

    with tile.TileContext(nc) as tc:
        with (
            tc.tile_pool(name="persist", bufs=1) as pp,
            tc.tile_pool(name="wstream", bufs=3) as wp,
            tc.tile_pool(name="work", bufs=4) as wk,
            tc.tile_pool(name="big1", bufs=2) as bg,
            tc.tile_pool(name="small", bufs=2) as sm,
            tc.tile_pool(name="sqtmp", bufs=1) as sq_p,
            tc.tile_pool(name="psA", bufs=1, space="PSUM") as psA,
            tc.tile_pool(name="psS", bufs=2, space="PSUM") as psS,
            tc.tile_pool(name="psT", bufs=2, space="PSUM") as psT,
            tc.tile_pool(name="dram", bufs=1, space="DRAM") as dr,
        ):
            # ---- persistent tiles ----
            xm = pp.tile([128, 32, B], bf16)
            xa = pp.tile([128, JC * I], bf16)
            xq = pp.tile([128, 2, I, 128], bf16)
            idm = pp.tile([128, 128], bf16)
            idf = pp.tile([128, 128], f32)
            sel = pp.tile([128, B], bf16)
            rep = pp.tile([B, 128], bf16)
            ond = pp.tile([128, 4], f32)
            rp4 = pp.tile([4, 128], f32)
            for t_, d_ in ((xm, xm_d), (xa, xa_d), (xq, xq_d), (idm, id_d),
                           (idf, idf_d), (sel, sel_d), (rep, rep_d),
                           (ond, ond_d), (rp4, rp4_d)):
                nc.sync.dma_start(t_[:], d_[:])

            logits = pp.tile([128, NCH, 256], bf16)
            v_bd = pp.tile([128, NCH, 128], bf16)   # blockdiag stationaries
            c_T = pp.tile([128, 2, NCH, 128], bf16)  # [(j), jt, ch, (q,b)]
            s_sb = pp.tile([128, NCH, B], f32)       # [(q,d), ch, b]
            s_full = pp.tile([128, NCH, B], f32)
            eps_t = pp.tile([128, 1], f32)
            warm = pp.tile([128, 8], f32)
            nc.vector.memset(eps_t[:], EPS)
            nc.vector.memset(v_bd[:], 0.0)
            nc.vector.memset(logits[:], 0.0)
            nc.vector.memset(warm[:], 0.0)

            def pe_t(out_ps, in_sb, k, ident):
                nc.tensor.matmul(out_ps, in_sb, ident[:k, :k],
                                 is_transpose=True, start=True, stop=True,
                                 skip_group_check=True)

            def allreduce(sbuf_in, sbuf_out, cols):
                ci = dr.tile([128, cols], f32)
                co = dr.tile([128, cols], f32)
                nc.sync.dma_start(ci[:], sbuf_in)
                nc.gpsimd.collective_compute(
                    "AllReduce", ADD,
                    replica_groups=[list(range(NCORES))],
                    ins=[ci[:].opt()], outs=[co[:].opt()],
                )
                nc.sync.dma_start(sbuf_out, co[:])

            # warmup collective: absorbs CC ring setup under r0 streaming
            allreduce(warm[:], warm[:], 8)

            def squash_update_v():
                """squash in [(q,d),(ch,b)] layout; fill v_bd diagonals."""
                sq = sq_p.tile([128, NCH * B], f32)
                sf = s_full[:].rearrange("p a b -> p (a b)")
                nc.vector.tensor_tensor(sq[:], sf, sf, op=MUL)
                ns2_ps = psT.tile([4, NCH * B], f32, tag="pst")
                nc.tensor.matmul(ns2_ps[:], ond[:], sq[:],
                                 start=True, stop=True, skip_group_check=True)
                ns2 = sq_p.tile([4, NCH * B], f32)
                nc.vector.tensor_copy(ns2[:], ns2_ps[:])
                onep = sq_p.tile([4, NCH * B], f32)
                nc.vector.tensor_scalar_add(onep[:], ns2[:], 1.0)
                rt = sq_p.tile([4, NCH * B], f32)
                nc.scalar.activation(rt[:], ns2[:], AF.Sqrt, bias=eps_t[:4, :])
                den = sq_p.tile([4, NCH * B], f32)
                nc.vector.tensor_tensor(den[:], onep[:], rt[:], op=MUL)
                dinv = sq_p.tile([4, NCH * B], f32)
                nc.vector.reciprocal(dinv[:], den[:])
                scl = sq_p.tile([4, NCH * B], f32)
                nc.vector.tensor_tensor(scl[:], ns2[:], dinv[:], op=MUL)
                sr_ps = psT.tile([128, NCH * B], f32, tag="pst")
                nc.tensor.matmul(sr_ps[:], rp4[:], scl[:],
                                 start=True, stop=True, skip_group_check=True)
                v_sb = sq_p.tile([128, NCH, B], bf16)
                nc.vector.tensor_tensor(
                    v_sb[:].rearrange("p a b -> p (a b)"), sf, sr_ps[:],
                    op=MUL)
                for q in range(4):
                    nc.vector.tensor_copy(
                        v_bd[32 * q:32 * q + 32, :, 32 * q:32 * q + 32],
                        v_sb[32 * q:32 * q + 32, :, :])

            # ================= r0 =================
            for cch in range(4):
                acc = psS.tile([B, 512], f32, tag="pss")
                dma_engs = ([nc.scalar, nc.gpsimd] if cch == 0 else
                            [nc.scalar, nc.gpsimd, nc.sync])
                for g in range(8):
                    wm_t = wp.tile([128, 2048], bf16, name="wm_t")
                    dma_engs[g % len(dma_engs)].dma_start(wm_t[:],
                                                          wm_d[cch, g])
                    for tt in range(4):
                        nc.tensor.matmul(
                            acc[:], xm[:, 4 * g + tt, :],
                            wm_t[:, 512 * tt:512 * tt + 512],
                            start=(g == 0 and tt == 0),
                            stop=(g == 7 and tt == 3),
                            skip_group_check=True)
                s0c = sm.tile([B, 512], bf16)
                nc.scalar.activation(s0c[:], acc[:], AF.Copy, scale=1.0 / N)
                for c4 in range(4):
                    ch = 4 * cch + c4
                    tp = psT.tile([128, B], bf16, tag="pst")
                    pe_t(tp[:], s0c[:, 128 * c4:128 * c4 + 128], B, idm)
                    nc.vector.tensor_copy(s_sb[:, ch, :], tp[:])
            allreduce(s_sb[:].rearrange("p a b -> p (a b)"),
                      s_full[:].rearrange("p a b -> p (a b)"), NCH * B)
            if dbg:
                nc.sync.dma_start(dbg_s0[:],
                                  s_full[:].rearrange("p a b -> p (a b)"))
            squash_update_v()

            # ================= r1, r2 =================
            for r in range(1, ROUTINGS):
                sacc = [psS.tile([128, 8, 128], f32, tag="pss",
                                 name=f"sacc{cb}") for cb in range(2)]
                for cb in range(2):
                    nc.vector.memset(sacc[cb][:], 0.0)

                def emit_A(ch, jt, eng_dma):
                    """A-phase for (ch, j-half jt); updates logits slice."""
                    wa_t = wp.tile([128, 2048], bf16, name="wa_t")
                    eng_dma.dma_start(wa_t[:],
                                      wa_d[ch][:, 2048 * jt:2048 * jt + 2048])
                    for qq in range(2):
                        aps = psA.tile([128, 1024], f32)
                        for cc in range(2):
                            nc.tensor.matmul(
                                aps[:, 512 * cc:512 * cc + 512],
                                v_bd[:, ch, :],
                                wa_t[:, 1024 * qq + 512 * cc:
                                     1024 * qq + 512 * cc + 512],
                                start=True, stop=True, skip_group_check=True)
                        p0 = wk.tile([128, 1024], bf16)
                        nc.scalar.activation(p0[:], aps[:], AF.Copy)
                        p1 = wk.tile([128, 64, 16], bf16)
                        o = 2048 * jt + 1024 * qq
                        nc.vector.tensor_tensor(
                            p1[:].rearrange("p a b -> p (a b)"), p0[:],
                            xa[:, o:o + 1024], op=MUL)
                        t1 = wk.tile([128, 64, 8], bf16)
                        nc.gpsimd.tensor_tensor(t1[:], p1[:, :, 0:8],
                                                p1[:, :, 8:16], op=ADD)
                        t2 = sm.tile([128, 64, 4], bf16)
                        nc.vector.tensor_tensor(t2[:], t1[:, :, 0:4],
                                                t1[:, :, 4:8], op=ADD)
                        t3 = sm.tile([128, 64, 2], bf16)
                        nc.vector.tensor_tensor(t3[:], t2[:, :, 0:2],
                                                t2[:, :, 2:4], op=ADD)
                        t4 = sm.tile([128, 64], bf16)
                        nc.vector.tensor_tensor(t4[:], t3[:, :, 0],
                                                t3[:, :, 1], op=ADD)
                        jo = 128 * jt + 64 * qq
                        with nc.allow_low_precision("bf16 routing logits"):
                            nc.vector.tensor_tensor(
                                logits[:, ch, jo:jo + 64],
                                logits[:, ch, jo:jo + 64], t4[:], op=ADD)

                def emit_softmax_ct(jt):
                    """softmax over n on j-half jt; write c_T[:, jt]."""
                    eeh = bg.tile([128, NCH, 128], bf16, name="eeh")
                    nc.scalar.activation(eeh[:],
                                         logits[:, :, 128 * jt:128 * jt + 128],
                                         AF.Exp)
                    e1 = sm.tile([128, 8, 128], bf16)
                    nc.vector.tensor_tensor(e1[:], eeh[:, 0:8, :],
                                            eeh[:, 8:16, :], op=ADD)
                    e2 = sm.tile([128, 4, 128], bf16)
                    nc.vector.tensor_tensor(e2[:], e1[:, 0:4, :],
                                            e1[:, 4:8, :], op=ADD)
                    e3 = sm.tile([128, 2, 128], bf16)
                    nc.vector.tensor_tensor(e3[:], e2[:, 0:2, :],
                                            e2[:, 2:4, :], op=ADD)
                    e4 = sm.tile([128, 128], bf16)
                    nc.vector.tensor_tensor(e4[:], e3[:, 0, :], e3[:, 1, :],
                                            op=ADD)
                    z_ps = psT.tile([B, 128], f32, tag="pst")
                    nc.tensor.matmul(z_ps[:], sel[:], e4[:],
                                     start=True, stop=True,
                                     skip_group_check=True)
                    zrec = sm.tile([B, 128], f32)
                    nc.vector.reciprocal(zrec[:], z_ps[:])
                    zrecb = sm.tile([B, 128], bf16)
                    nc.vector.tensor_copy(zrecb[:], zrec[:])
                    zr_ps = psT.tile([128, 128], f32, tag="pst")
                    nc.tensor.matmul(zr_ps[:], rep[:], zrecb[:],
                                     start=True, stop=True,
                                     skip_group_check=True)
                    zr = sm.tile([128, 128], bf16)
                    nc.scalar.activation(zr[:], zr_ps[:], AF.Copy)
                    ch_t = bg.tile([128, NCH, 128], bf16, name="ch_t")
                    nc.vector.tensor_tensor(
                        ch_t[:], eeh[:],
                        zr[:, None, :].broadcast_to([128, NCH, 128]), op=MUL)
                    for ch in range(NCH):
                        ctp = psT.tile([128, 128], bf16, tag="pst")
                        pe_t(ctp[:], ch_t[:, ch, :], 128, idm)
                        nc.scalar.activation(c_T[:, jt, ch, :], ctp[:],
                                             AF.Copy)

                ws_cache = {}

                def emit_s(jt, i, eng_dma):
                    """s-matmuls for all 16 ch at (i, jt)."""
                    ig, i2 = i // 2, i % 2
                    for chB in range(2):
                        key = (jt, ig, chB)
                        if key not in ws_cache:
                            w = wp.tile([128, 2048], bf16, name="ws_t")
                            eng_dma.dma_start(w[:], ws_d[chB, jt, ig])
                            ws_cache[key] = w
                        ws_t = ws_cache[key]
                        y_t = wk.tile([128, 8, 128], bf16, name="y_t")
                        nc.vector.tensor_tensor(
                            y_t[:],
                            c_T[:, jt, 8 * chB:8 * chB + 8, :],
                            xq[:, jt, i, None, :]
                            .broadcast_to([128, 8, 128]), op=MUL)
                        last = (i == I - 1 and jt == 1)
                        for c8 in range(8):
                            nc.tensor.matmul(
                                sacc[chB][:, c8, :],
                                ws_t[:, 1024 * i2 + 128 * c8:
                                     1024 * i2 + 128 * c8 + 128],
                                y_t[:, c8, :],
                                start=False, stop=last,
                                skip_group_check=True)

                # --- jt=0 A-phase ---
                for ch in range(NCH):
                    emit_A(ch, 0, nc.scalar)
                emit_softmax_ct(0)
                # --- pipeline: A(jt=1) interleaved with s(jt=0) ---
                for ch in range(NCH):
                    emit_A(ch, 1, nc.scalar)
                    emit_s(0, ch, nc.gpsimd)
                emit_softmax_ct(1)
                # --- s(jt=1) ---
                for i in range(I):
                    emit_s(1, i, nc.gpsimd)
                ws_cache.clear()

                # --- extract diagonal blocks, AllReduce ---
                for chB in range(2):
                    for c8 in range(8):
                        for q in range(4):
                            nc.vector.tensor_copy(
                                s_sb[32 * q:32 * q + 32, 8 * chB + c8, :],
                                sacc[chB][32 * q:32 * q + 32, c8,
                                          32 * q:32 * q + 32])
                if dbg and r == 1:
                    nc.sync.dma_start(
                        dbg_lg[:], logits[:].rearrange("p a b -> p (a b)"))
                    nc.sync.dma_start(dbg_sp[:],
                                      s_sb[:].rearrange("p a b -> p (a b)"))
                allreduce(s_sb[:].rearrange("p a b -> p (a b)"),
                          s_full[:].rearrange("p a b -> p (a b)"), NCH * B)

                if r < ROUTINGS - 1:
                    squash_update_v()
                else:
                    v_out = sm.tile([128, 4, 128], f32)
                    for k4 in range(4):
                        stp = psT.tile([128, 128], f32, tag="pst")
                        pe_t(stp[:],
                             s_full[:, 4 * k4:4 * k4 + 4, :]
                             .rearrange("p a b -> p (a b)"), 128, idf)
                        sT = sm.tile([128, 4, D], f32)
                        nc.vector.tensor_copy(
                            sT[:].rearrange("p a b -> p (a b)"), stp[:])
                        sq2 = sm.tile([128, 4, D], f32)
                        nc.vector.tensor_tensor(sq2[:], sT[:], sT[:], op=MUL)
                        ns2 = sq_p.tile([128, 4], f32)
                        nc.vector.tensor_reduce(ns2[:], sq2[:], axis=AX,
                                                op=ADD)
                        onep = sq_p.tile([128, 4], f32)
                        nc.vector.tensor_scalar_add(onep[:], ns2[:], 1.0)
                        rt = sq_p.tile([128, 4], f32)
                        nc.scalar.activation(rt[:], ns2[:], AF.Sqrt,
                                             bias=eps_t[:])
                        den = sq_p.tile([128, 4], f32)
                        nc.vector.tensor_tensor(den[:], onep[:], rt[:],
                                                op=MUL)
                        dinv = sq_p.tile([128, 4], f32)
                        nc.vector.reciprocal(dinv[:], den[:])
                        scl = sq_p.tile([128, 4], f32)
                        nc.vector.tensor_tensor(scl[:], ns2[:], dinv[:],
                                                op=MUL)
                        nc.vector.tensor_tensor(
                            v_out[:, k4, :].rearrange("p (a b) -> p a b", b=D),
                            sT[:],
                            scl[:, :, None].broadcast_to([128, 4, D]), op=MUL)
                    vd = v_d[:].rearrange("b (k f) -> b k f", f=128)
                    for k4 in range(4):
                        for chm in range(4):
                            nc.sync.dma_start(
                                vd[:, 4 * k4 + chm, :],
                                v_out[32 * chm:32 * chm + 32, k4, :])

    nc.compile()
    return nc


def make_in_maps(x: np.ndarray, W: np.ndarray):
    import ml_dtypes
    bf = ml_dtypes.bfloat16
    in_maps = []
    idm = np.eye(128, dtype=np.float32).astype(bf)
    idf = np.eye(128, dtype=np.float32)
    sel = np.tile(np.eye(B, dtype=np.float32), (4, 1)).astype(bf)
    rep = np.tile(np.eye(B, dtype=np.float32), (1, 4)).astype(bf)
    ond = np.kron(np.eye(4, dtype=np.float32), np.ones((32, 1), np.float32))
    rp4 = np.kron(np.eye(4, dtype=np.float32), np.ones((1, 32), np.float32))
    for k in range(NCORES):
        Wk = np.ascontiguousarray(W[:, k * JC:(k + 1) * JC])  # [64,256,32,16]
        xk = np.ascontiguousarray(x[:, k * JC:(k + 1) * JC])  # [32,256,16]
        # wm[cch, g, (8jr,16i), (tt,512)] = Wk[n, 8*(4g+tt)+jr, d, i]
        t1 = Wk.transpose(1, 3, 0, 2).reshape(32, 8, 16, 4, 512)
        wm = t1.transpose(3, 0, 1, 2, 4).reshape(4, 32, 128, 512)
        wm = np.ascontiguousarray(
            wm.reshape(4, 8, 4, 128, 512).transpose(0, 1, 3, 2, 4)
            .reshape(4, 8, 128, 2048))
        # xm[(jr,i), t, b] = xk[b, 8t+jr, i]
        xm = np.ascontiguousarray(
            xk.transpose(1, 2, 0).reshape(32, 8, 16, B)
            .transpose(1, 2, 0, 3).reshape(128, 32, B))
        # wa[ch, (q,d), (j,i)] = Wk[4ch+q, j, d, i]
        wa = np.ascontiguousarray(
            Wk.transpose(0, 2, 1, 3).reshape(NCH, 128, JC * I))
        # xa[(q,b), (j,i)] = xk[b, j, i], replicated over q
        xa = np.ascontiguousarray(np.tile(xk.reshape(B, JC * I), (4, 1)))
        # ws[chB, jt, ig, j, (i2,c8,q,d)] = Wk[4*(8chB+c8)+q, 128jt+j, d, i]
        t2 = Wk.transpose(3, 1, 0, 2)             # [i, j, n, d]
        t2 = t2.reshape(8, 2, 2, 128, 2, 8, 128)  # [ig,i2,jt,j,chB,c8,(q,d)]
        ws = np.ascontiguousarray(
            t2.transpose(4, 2, 0, 3, 1, 5, 6).reshape(2, 2, 8, 128, 2048))
        # xq[j, jt, i, (q,b)] = xk[b, 128jt+j, i] replicated over q
        xqa = xk.transpose(1, 2, 0).reshape(2, 128, 16, B)  # [jt, j, i, b]
        xqa = np.broadcast_to(xqa[:, :, :, None, :], (2, 128, 16, 4, B))
        xqa = np.ascontiguousarray(
            xqa.reshape(2, 128, 16, 128).transpose(1, 0, 2, 3))
        in_maps.append({
            "wm": wm.astype(bf), "xm": xm.astype(bf),
            "wa": wa.astype(bf), "xa": xa.astype(bf),
            "ws": ws.astype(bf), "xq": np.ascontiguousarray(xqa).astype(bf),
            "idm": idm, "idf": idf, "sel": sel, "rep": rep,
            "ond": ond, "rp4": rp4,
        })
    return in_maps


def kernel(x: np.ndarray, W: np.ndarray) -> np.ndarray:
    from concourse.bass_utils import run_bass_kernel_spmd

    nc = _build()
    in_maps = make_in_maps(x, W)
    res = run_bass_kernel_spmd(nc, in_maps, list(range(NCORES)))
    v = np.asarray(res.results[0]["v"], dtype=np.float32)
    return v.reshape(B, N, D)


if __name__ == "__main__":
    rng = np.random.default_rng(0)
    x = rng.normal(size=(B, J, I)).astype(np.float32)
    W = rng.normal(size=(N, J, D, I)).astype(np.float32) * 0.05
    v = kernel(x, W)
    print(v.shape, v.dtype, np.abs(v).max())


# revision 19
# speedup vs baseline: 1.2085x; 1.2085x over previous
"""CapsuleLayer dynamic-routing kernel for 8 Trainium2 NeuronCores, v3.

Math (reference):
    u_hat[b,n,j,d] = sum_i W[n,j,d,i] * x[b,j,i]
    b = 0; for r in 0..2:
        c = softmax_n(b); s[b,n,d] = sum_j c*u_hat; v = squash_d(s)
        if r < 2: b += sum_d v*u_hat
    return v  [B, N, D]

Design: u_hat is NEVER materialized (its per-j matmul only uses K=16 of
the 128 PE rows). All big PE work runs at K=128:

  r0:   s0[b,(n,d)] = (1/N) sum_{(j,i)} x[b,(j,i)] W[(j,i),(n,d)]
        -> dense matmuls, stationary x K-tiles [(8j,16i),b].
  r>=1: dlog[b,n,j] = sum_i A[b,n,j,i] x[b,j,i],
        A[b,n,j,i] = sum_d v[b,n,d] W[n,j,d,i] computed by PE with a
        block-diagonal stationary blockdiag(v_n^T for 4 n) [128,128]
        and moving W_A[(q,d),(j,i)] -> out [(q,b),(j,i)] at K=128.
        The i-reduction is a DVE mult + add-tree (L1 on GpSimd).
  r>=1: s via y = c (x) x folded: stationary W_S[(j),(c8:q,d)] chunks,
        moving y^T[(j),(q,b)] computed directly in j-partition layout
        from a PE-transposed c. Diagonal 4x4 blocks of the [128,128]
        outputs are the s partials, accumulated in PSUM over (i, jt).
        PSUM accumulators are memset-initialized and all matmuls use
        start=False (start=True resets the whole PSUM bank, which
        clobbers co-located accumulator regions).

Pipelining: softmax over n only needs all capsules for a FIXED j, so
the iteration is split into j-halves (jt): while the s-matmuls of
jt=0 run on the PE, the A-phase of jt=1 runs concurrently (its DVE /
GpSimd reduction work hides under the PE stream). A tiny warmup
AllReduce absorbs the collective's ring-setup cost during r0's weight
streaming, and big weight DMAs are spread over several engine queues.

Sharding: J (2048) split 8 ways -> Jc=256/core; s AllReduced in f32.
"""

import functools
import numpy as np

B, J, I = 32, 2048, 16
N, D = 64, 32
NCORES = 8
JC = J // NCORES          # 256 j per core
ND = N * D                # 2048
NCH = 16                  # n-blocks of 4 (ch); n = 4*ch + q
ROUTINGS = 3
EPS = 1e-7


@functools.lru_cache(maxsize=2)
def _build(dbg=False):
    import concourse.bass as bass
    import concourse.mybir as mybir
    import concourse.bacc as bacc
    import concourse.tile as tile

    f32 = mybir.dt.float32
    bf16 = mybir.dt.bfloat16
    MUL = mybir.AluOpType.mult
    ADD = mybir.AluOpType.add
    AX = mybir.AxisListType.X
    AF = mybir.ActivationFunctionType

    nc = bacc.Bacc("TRN2", target_bir_lowering=False, debug=False,
                   num_devices=NCORES)

    # --- DRAM inputs (per-core shard layouts, prepared host-side) ---
    # r0: wm[cch, g, (8jr,16i), (4tt,512)]  (t = 4g+tt)
    wm_d = nc.dram_tensor("wm", [4, 8, 128, 2048], bf16, kind="ExternalInput")
    xm_d = nc.dram_tensor("xm", [128, 32, B], bf16, kind="ExternalInput")
    wa_d = nc.dram_tensor("wa", [NCH, 128, JC * I], bf16, kind="ExternalInput")
    xa_d = nc.dram_tensor("xa", [128, JC * I], bf16, kind="ExternalInput")
    # s-phase: ws[chB, jt, ig, (j), (i2,c8,q,d)]  (i = 2*ig+i2)
    ws_d = nc.dram_tensor("ws", [2, 2, 8, 128, 2048], bf16,
                          kind="ExternalInput")
    # xq[(j), jt, i, (q,b)] : x replicated over q
    xq_d = nc.dram_tensor("xq", [128, 2, I, 128], bf16, kind="ExternalInput")
    id_d = nc.dram_tensor("idm", [128, 128], bf16, kind="ExternalInput")
    idf_d = nc.dram_tensor("idf", [128, 128], f32, kind="ExternalInput")
    sel_d = nc.dram_tensor("sel", [128, B], bf16, kind="ExternalInput")
    rep_d = nc.dram_tensor("rep", [B, 128], bf16, kind="ExternalInput")
    ond_d = nc.dram_tensor("ond", [128, 4], f32, kind="ExternalInput")
    rp4_d = nc.dram_tensor("rp4", [4, 128], f32, kind="ExternalInput")
    v_d = nc.dram_tensor("v", [B, ND], f32, kind="ExternalOutput")
    if dbg:
        dbg_s0 = nc.dram_tensor("dbg_s0", [128, NCH * B], f32,
                                kind="ExternalOutput")
        dbg_lg = nc.dram_tensor("dbg_lg", [128, NCH * 256], bf16,
                                kind="ExternalOutput")
        dbg_sp = nc.dram_tensor("dbg_sp", [128, NCH * B], f32,
                                kind="ExternalOutput")

    with tile.TileContext(nc) as tc:
        with (
            tc.tile_pool(name="persist", bufs=1) as pp,
            tc.tile_pool(name="wstream", bufs=3) as wp,
            tc.tile_pool(name="work", bufs=4) as wk,
            tc.tile_pool(name="big1", bufs=2) as bg,
            tc.tile_pool(name="small", bufs=2) as sm,
            tc.tile_pool(name="sqtmp", bufs=1) as sq_p,
            tc.tile_pool(name="psA", bufs=2, space="PSUM") as psA,
            tc.tile_pool(name="psS", bufs=2, space="PSUM") as psS,
            tc.tile_pool(name="psT", bufs=2, space="PSUM") as psT,
            tc.tile_pool(name="dram", bufs=1, space="DRAM") as dr,
        ):
            # ---- persistent tiles ----
            xm = pp.tile([128, 32, B], bf16)
            xa = pp.tile([128, JC * I], bf16)
            xq = pp.tile([128, 2, I, 128], bf16)
            idm = pp.tile([128, 128], bf16)
            idf = pp.tile([128, 128], f32)
            sel = pp.tile([128, B], bf16)
            rep = pp.tile([B, 128], bf16)
            ond = pp.tile([128, 4], f32)
            rp4 = pp.tile([4, 128], f32)
            for t_, d_ in ((xm, xm_d), (xa, xa_d), (xq, xq_d), (idm, id_d),
                           (idf, idf_d), (sel, sel_d), (rep, rep_d),
                           (ond, ond_d), (rp4, rp4_d)):
                nc.sync.dma_start(t_[:], d_[:])

            logits = pp.tile([128, NCH, 256], bf16)
            v_bd = pp.tile([128, NCH, 128], bf16)   # blockdiag stationaries
            c_T = pp.tile([128, 2, NCH, 128], bf16)  # [(j), jt, ch, (q,b)]
            s_sb = pp.tile([128, NCH, B], f32)       # [(q,d), ch, b]
            s_full = pp.tile([128, NCH, B], f32)
            eps_t = pp.tile([128, 1], f32)
            nc.vector.memset(eps_t[:], EPS)
            nc.vector.memset(v_bd[:], 0.0)
            nc.vector.memset(logits[:], 0.0)

            def pe_t(out_ps, in_sb, k, ident):
                nc.tensor.matmul(out_ps, in_sb, ident[:k, :k],
                                 is_transpose=True, start=True, stop=True,
                                 skip_group_check=True)

            def allreduce(sbuf_in, sbuf_out, cols):
                ci = dr.tile([128, cols], f32)
                co = dr.tile([128, cols], f32)
                nc.sync.dma_start(ci[:], sbuf_in)
                nc.gpsimd.collective_compute(
                    "AllReduce", ADD,
                    replica_groups=[list(range(NCORES))],
                    ins=[ci[:].opt()], outs=[co[:].opt()],
                )
                nc.sync.dma_start(sbuf_out, co[:])

            def squash_update_v():
                """squash in [(q,d),(ch,b)] layout; fill v_bd diagonals."""
                sq = sq_p.tile([128, NCH * B], f32)
                sf = s_full[:].rearrange("p a b -> p (a b)")
                nc.vector.tensor_tensor(sq[:], sf, sf, op=MUL)
                ns2_ps = psT.tile([4, NCH * B], f32, tag="pst")
                nc.tensor.matmul(ns2_ps[:], ond[:], sq[:],
                                 start=True, stop=True, skip_group_check=True)
                ns2 = sq_p.tile([4, NCH * B], f32)
                nc.vector.tensor_copy(ns2[:], ns2_ps[:])
                onep = sq_p.tile([4, NCH * B], f32)
                nc.vector.tensor_scalar_add(onep[:], ns2[:], 1.0)
                rt = sq_p.tile([4, NCH * B], f32)
                nc.scalar.activation(rt[:], ns2[:], AF.Sqrt, bias=eps_t[:4, :])
                den = sq_p.tile([4, NCH * B], f32)
                nc.vector.tensor_tensor(den[:], onep[:], rt[:], op=MUL)
                dinv = sq_p.tile([4, NCH * B], f32)
                nc.vector.reciprocal(dinv[:], den[:])
                scl = sq_p.tile([4, NCH * B], f32)
                nc.vector.tensor_tensor(scl[:], ns2[:], dinv[:], op=MUL)
                sr_ps = psT.tile([128, NCH * B], f32, tag="pst")
                nc.tensor.matmul(sr_ps[:], rp4[:], scl[:],
                                 start=True, stop=True, skip_group_check=True)
                v_sb = sq_p.tile([128, NCH, B], bf16)
                nc.vector.tensor_tensor(
                    v_sb[:].rearrange("p a b -> p (a b)"), sf, sr_ps[:],
                    op=MUL)
                for q in range(4):
                    nc.vector.tensor_copy(
                        v_bd[32 * q:32 * q + 32, :, 32 * q:32 * q + 32],
                        v_sb[32 * q:32 * q + 32, :, :])

            # ================= r0 =================
            for cch in range(4):
                acc = psS.tile([B, 512], f32, tag="pss")
                dma_engs = ([nc.scalar, nc.gpsimd] if cch == 0 else
                            [nc.scalar, nc.gpsimd, nc.sync])
                for g in range(8):
                    wm_t = wp.tile([128, 2048], bf16, name="wm_t")
                    dma_engs[g % len(dma_engs)].dma_start(wm_t[:],
                                                          wm_d[cch, g])
                    for tt in range(4):
                        nc.tensor.matmul(
                            acc[:], xm[:, 4 * g + tt, :],
                            wm_t[:, 512 * tt:512 * tt + 512],
                            start=(g == 0 and tt == 0),
                            stop=(g == 7 and tt == 3),
                            skip_group_check=True)
                s0c = sm.tile([B, 512], bf16)
                nc.scalar.activation(s0c[:], acc[:], AF.Copy, scale=1.0 / N)
                for c4 in range(4):
                    ch = 4 * cch + c4
                    tp = psT.tile([128, B], bf16, tag="pst")
                    pe_t(tp[:], s0c[:, 128 * c4:128 * c4 + 128], B, idm)
                    nc.vector.tensor_copy(s_sb[:, ch, :], tp[:])
            allreduce(s_sb[:].rearrange("p a b -> p (a b)"),
                      s_full[:].rearrange("p a b -> p (a b)"), NCH * B)
            if dbg:
                nc.sync.dma_start(dbg_s0[:],
                                  s_full[:].rearrange("p a b -> p (a b)"))
            squash_update_v()

            # ================= r1, r2 =================
            for r in range(1, ROUTINGS):
                sacc = [psS.tile([128, 8, 128], f32, tag="pss",
                                 name=f"sacc{cb}") for cb in range(2)]
                for cb in range(2):
                    nc.vector.memset(sacc[cb][:], 0.0)

                def emit_A(ch, jt, eng_dma):
                    """A-phase for (ch, j-half jt); updates logits slice."""
                    wa_t = wp.tile([128, 2048], bf16, name="wa_t")
                    eng_dma.dma_start(wa_t[:],
                                      wa_d[ch][:, 2048 * jt:2048 * jt + 2048])
                    for qq in range(4):
                        aps = psA.tile([128, 512], f32)
                        nc.tensor.matmul(
                            aps[:], v_bd[:, ch, :],
                            wa_t[:, 512 * qq:512 * qq + 512],
                            start=True, stop=True, skip_group_check=True)
                        p0 = wk.tile([128, 512], bf16)
                        nc.scalar.activation(p0[:], aps[:], AF.Copy)
                        p1 = wk.tile([128, 32, 16], bf16)
                        o = 2048 * jt + 512 * qq
                        nc.vector.tensor_tensor(
                            p1[:].rearrange("p a b -> p (a b)"), p0[:],
                            xa[:, o:o + 512], op=MUL)
                        t1 = wk.tile([128, 32, 8], bf16)
                        eng = nc.gpsimd if qq % 2 == 0 else nc.vector
                        eng.tensor_tensor(t1[:], p1[:, :, 0:8],
                                          p1[:, :, 8:16], op=ADD)
                        t2 = sm.tile([128, 32, 4], bf16)
                        nc.vector.tensor_tensor(t2[:], t1[:, :, 0:4],
                                                t1[:, :, 4:8], op=ADD)
                        t3 = sm.tile([128, 32, 2], bf16)
                        nc.vector.tensor_tensor(t3[:], t2[:, :, 0:2],
                                                t2[:, :, 2:4], op=ADD)
                        t4 = sm.tile([128, 32], bf16)
                        nc.vector.tensor_tensor(t4[:], t3[:, :, 0],
                                                t3[:, :, 1], op=ADD)
                        jo = 128 * jt + 32 * qq
                        with nc.allow_low_precision("bf16 routing logits"):
                            nc.vector.tensor_tensor(
                                logits[:, ch, jo:jo + 32],
                                logits[:, ch, jo:jo + 32], t4[:], op=ADD)

                def emit_softmax_ct(jt):
                    """softmax over n on j-half jt; write c_T[:, jt]."""
                    eeh = bg.tile([128, NCH, 128], bf16, name="eeh")
                    nc.scalar.activation(eeh[:],
                                         logits[:, :, 128 * jt:128 * jt + 128],
                                         AF.Exp)
                    e1 = sm.tile([128, 8, 128], bf16)
                    nc.vector.tensor_tensor(e1[:], eeh[:, 0:8, :],
                                            eeh[:, 8:16, :], op=ADD)
                    e2 = sm.tile([128, 4, 128], bf16)
                    nc.vector.tensor_tensor(e2[:], e1[:, 0:4, :],
                                            e1[:, 4:8, :], op=ADD)
                    e3 = sm.tile([128, 2, 128], bf16)
                    nc.vector.tensor_tensor(e3[:], e2[:, 0:2, :],
                                            e2[:, 2:4, :], op=ADD)
                    e4 = sm.tile([128, 128], bf16)
                    nc.vector.tensor_tensor(e4[:], e3[:, 0, :], e3[:, 1, :],
                                            op=ADD)
                    z_ps = psT.tile([B, 128], f32, tag="pst")
                    nc.tensor.matmul(z_ps[:], sel[:], e4[:],
                                     start=True, stop=True,
                                     skip_group_check=True)
                    zrec = sm.tile([B, 128], f32)
                    nc.vector.reciprocal(zrec[:], z_ps[:])
                    zrecb = sm.tile([B, 128], bf16)
                    nc.vector.tensor_copy(zrecb[:], zrec[:])
                    zr_ps = psT.tile([128, 128], f32, tag="pst")
                    nc.tensor.matmul(zr_ps[:], rep[:], zrecb[:],
                                     start=True, stop=True,
                                     skip_group_check=True)
                    zr = sm.tile([128, 128], bf16)
                    nc.scalar.activation(zr[:], zr_ps[:], AF.Copy)
                    ch_t = bg.tile([128, NCH, 128], bf16, name="ch_t")
                    nc.vector.tensor_tensor(
                        ch_t[:], eeh[:],
                        zr[:, None, :].broadcast_to([128, NCH, 128]), op=MUL)
                    for ch in range(NCH):
                        ctp = psT.tile([128, 128], bf16, tag="pst")
                        pe_t(ctp[:], ch_t[:, ch, :], 128, idm)
                        nc.scalar.activation(c_T[:, jt, ch, :], ctp[:],
                                             AF.Copy)

                ws_cache = {}

                def emit_s(jt, i, eng_dma):
                    """s-matmuls for all 16 ch at (i, jt)."""
                    ig, i2 = i // 2, i % 2
                    for chB in range(2):
                        key = (jt, ig, chB)
                        if key not in ws_cache:
                            w = wp.tile([128, 2048], bf16, name="ws_t")
                            eng_dma.dma_start(w[:], ws_d[chB, jt, ig])
                            ws_cache[key] = w
                        ws_t = ws_cache[key]
                        y_t = wk.tile([128, 8, 128], bf16, name="y_t")
                        nc.vector.tensor_tensor(
                            y_t[:],
                            c_T[:, jt, 8 * chB:8 * chB + 8, :],
                            xq[:, jt, i, None, :]
                            .broadcast_to([128, 8, 128]), op=MUL)
                        last = (i == I - 1 and jt == 1)
                        for c8 in range(8):
                            nc.tensor.matmul(
                                sacc[chB][:, c8, :],
                                ws_t[:, 1024 * i2 + 128 * c8:
                                     1024 * i2 + 128 * c8 + 128],
                                y_t[:, c8, :],
                                start=False, stop=last,
                                skip_group_check=True)

                # --- jt=0 A-phase ---
                for ch in range(NCH):
                    emit_A(ch, 0, nc.scalar)
                emit_softmax_ct(0)
                # --- pipeline: A(jt=1) interleaved with s(jt=0) ---
                for ch in range(NCH):
                    emit_A(ch, 1, nc.scalar)
                    emit_s(0, ch, nc.gpsimd)
                emit_softmax_ct(1)
                # --- s(jt=1) ---
                for i in range(I):
                    emit_s(1, i, nc.gpsimd)
                ws_cache.clear()

                # --- extract diagonal blocks, AllReduce ---
                for chB in range(2):
                    for c8 in range(8):
                        for q in range(4):
                            nc.vector.tensor_copy(
                                s_sb[32 * q:32 * q + 32, 8 * chB + c8, :],
                                sacc[chB][32 * q:32 * q + 32, c8,
                                          32 * q:32 * q + 32])
                if dbg and r == 1:
                    nc.sync.dma_start(
                        dbg_lg[:], logits[:].rearrange("p a b -> p (a b)"))
                    nc.sync.dma_start(dbg_sp[:],
                                      s_sb[:].rearrange("p a b -> p (a b)"))
                allreduce(s_sb[:].rearrange("p a b -> p (a b)"),
                          s_full[:].rearrange("p a b -> p (a b)"), NCH * B)

                if r < ROUTINGS - 1:
                    squash_update_v()
                else:
                    v_out = sm.tile([128, 4, 128], f32)
                    for k4 in range(4):
                        stp = psT.tile([128, 128], f32, tag="pst")
                        pe_t(stp[:],
                             s_full[:, 4 * k4:4 * k4 + 4, :]
                             .rearrange("p a b -> p (a b)"), 128, idf)
                        sT = sm.tile([128, 4, D], f32)
                        nc.vector.tensor_copy(
                            sT[:].rearrange("p a b -> p (a b)"), stp[:])
                        sq2 = sm.tile([128, 4, D], f32)
                        nc.vector.tensor_tensor(sq2[:], sT[:], sT[:], op=MUL)
                        ns2 = sq_p.tile([128, 4], f32)
                        nc.vector.tensor_reduce(ns2[:], sq2[:], axis=AX,
                                                op=ADD)
                        onep = sq_p.tile([128, 4], f32)
                        nc.vector.tensor_scalar_add(onep[:], ns2[:], 1.0)
                        rt = sq_p.tile([128, 4], f32)
                        nc.scalar.activation(rt[:], ns2[:], AF.Sqrt,
                                             bias=eps_t[:])
                        den = sq_p.tile([128, 4], f32)
                        nc.vector.tensor_tensor(den[:], onep[:], rt[:],
                                                op=MUL)
                        dinv = sq_p.tile([128, 4], f32)
                        nc.vector.reciprocal(dinv[:], den[:])
                        scl = sq_p.tile([128, 4], f32)
                        nc.vector.tensor_tensor(scl[:], ns2[:], dinv[:],
                                                op=MUL)
                        nc.vector.tensor_tensor(
                            v_out[:, k4, :].rearrange("p (a b) -> p a b", b=D),
                            sT[:],
                            scl[:, :, None].broadcast_to([128, 4, D]), op=MUL)
                    vd = v_d[:].rearrange("b (k f) -> b k f", f=128)
                    for k4 in range(4):
                        for chm in range(4):
                            nc.sync.dma_start(
                                vd[:, 4 * k4 + chm, :],
                                v_out[32 * chm:32 * chm + 32, k4, :])

    nc.compile()
    return nc


def make_in_maps(x: np.ndarray, W: np.ndarray):
    import ml_dtypes
    bf = ml_dtypes.bfloat16
    in_maps = []
    idm = np.eye(128, dtype=np.float32).astype(bf)
    idf = np.eye(128, dtype=np.float32)
    sel = np.tile(np.eye(B, dtype=np.float32), (4, 1)).astype(bf)
    rep = np.tile(np.eye(B, dtype=np.float32), (1, 4)).astype(bf)
    ond = np.kron(np.eye(4, dtype=np.float32), np.ones((32, 1), np.float32))
    rp4 = np.kron(np.eye(4, dtype=np.float32), np.ones((1, 32), np.float32))
    for k in range(NCORES):
        Wk = np.ascontiguousarray(W[:, k * JC:(k + 1) * JC])  # [64,256,32,16]
        xk = np.ascontiguousarray(x[:, k * JC:(k + 1) * JC])  # [32,256,16]
        # wm[cch, g, (8jr,16i), (tt,512)] = Wk[n, 8*(4g+tt)+jr, d, i]
        t1 = Wk.transpose(1, 3, 0, 2).reshape(32, 8, 16, 4, 512)
        wm = t1.transpose(3, 0, 1, 2, 4).reshape(4, 32, 128, 512)
        wm = np.ascontiguousarray(
            wm.reshape(4, 8, 4, 128, 512).transpose(0, 1, 3, 2, 4)
            .reshape(4, 8, 128, 2048))
        # xm[(jr,i), t, b] = xk[b, 8t+jr, i]
        xm = np.ascontiguousarray(
            xk.transpose(1, 2, 0).reshape(32, 8, 16, B)
            .transpose(1, 2, 0, 3).reshape(128, 32, B))
        # wa[ch, (q,d), (j,i)] = Wk[4ch+q, j, d, i]
        wa = np.ascontiguousarray(
            Wk.transpose(0, 2, 1, 3).reshape(NCH, 128, JC * I))
        # xa[(q,b), (j,i)] = xk[b, j, i], replicated over q
        xa = np.ascontiguousarray(np.tile(xk.reshape(B, JC * I), (4, 1)))
        # ws[chB, jt, ig, j, (i2,c8,q,d)] = Wk[4*(8chB+c8)+q, 128jt+j, d, i]
        t2 = Wk.transpose(3, 1, 0, 2)             # [i, j, n, d]
        t2 = t2.reshape(8, 2, 2, 128, 2, 8, 128)  # [ig,i2,jt,j,chB,c8,(q,d)]
        ws = np.ascontiguousarray(
            t2.transpose(4, 2, 0, 3, 1, 5, 6).reshape(2, 2, 8, 128, 2048))
        # xq[j, jt, i, (q,b)] = xk[b, 128jt+j, i] replicated over q
        xqa = xk.transpose(1, 2, 0).reshape(2, 128, 16, B)  # [jt, j, i, b]
        xqa = np.broadcast_to(xqa[:, :, :, None, :], (2, 128, 16, 4, B))
        xqa = np.ascontiguousarray(
            xqa.reshape(2, 128, 16, 128).transpose(1, 0, 2, 3))
        in_maps.append({
            "wm": wm.astype(bf), "xm": xm.astype(bf),
            "wa": wa.astype(bf), "xa": xa.astype(bf),
            "ws": ws.astype(bf), "xq": np.ascontiguousarray(xqa).astype(bf),
            "idm": idm, "idf": idf, "sel": sel, "rep": rep,
            "ond": ond, "rp4": rp4,
        })
    return in_maps


def kernel(x: np.ndarray, W: np.ndarray) -> np.ndarray:
    from concourse.bass_utils import run_bass_kernel_spmd

    nc = _build()
    in_maps = make_in_maps(x, W)
    res = run_bass_kernel_spmd(nc, in_maps, list(range(NCORES)))
    v = np.asarray(res.results[0]["v"], dtype=np.float32)
    return v.reshape(B, N, D)


if __name__ == "__main__":
    rng = np.random.default_rng(0)
    x = rng.normal(size=(B, J, I)).astype(np.float32)
    W = rng.normal(size=(N, J, D, I)).astype(np.float32) * 0.05
    v = kernel(x, W)
    print(v.shape, v.dtype, np.abs(v).max())
